# revision 2
# baseline (speedup 1.0000x reference)
"""Bidirectional GRU Bass kernel for TRN2 — v2: direction-parallel sharding.

Problem: B=64, L=1024, IN=H=512, bidirectional GRU (torch GRUCell semantics),
mask = ones, output concat([lr, reversed(rl)], axis=2).

Sharding v2: 8 cores = 4 batch groups x 2 directions. Each core runs ONE
direction over B_SH=16 sequences. The rl cores receive time-reversed feats
(host-side flip) so the on-device program is identical SPMD; the host
re-reverses their output when assembling the full [64, L, 2H] tensor.

Per-core layout ("transposed domain"):
  - hidden state hTb[p, kc, b] = h[b, 128*kc + p]   (SBUF [128, 4, 32], bf16,
    cols 16:32 zero pad)
  - recurrent matmul: 4x col-tiled, stationary hTb, moving Whh (bf16),
    psum [128, 384]; 32x32 stream transpose into gate domain [128, 12, 32].
  - token partition order within an 8-token group g: p = 8*b + t.
"""

from contextlib import ExitStack

import numpy as np

import concourse.bass as bass
import concourse.mybir as mybir
import concourse.tile as tile
from concourse._compat import with_exitstack
from concourse.masks import make_identity

F32 = mybir.dt.float32
F32R = mybir.dt.float32r
BF16 = mybir.dt.bfloat16
F8E4 = mybir.dt.float8e4

IN = 512
H = 512
G = 3 * H  # 1536
KC = 4     # k chunks of 128 (contraction over H or IN)
MC = 12    # gate chunks of 128 (3H)
B_SH = 16  # batch per core (4 groups x 2 dirs over 8 cores)
TG = 8     # tokens per partition-group


def prep_inputs(feats, w_ih_lr, w_hh_lr, b_ih_lr, b_hh_lr,
                w_ih_rl, w_hh_rl, b_ih_rl, b_hh_rl, n_cores=8):
    """Host-side: shard feats over (batch-group, direction), arrange weights.

    Core c handles batch group c>>1 with direction c&1 (0=lr, 1=rl).
    rl cores get time-reversed feats so the device program is direction-free.
    """
    feats = np.asarray(feats, dtype=np.float32)

    def arrange_w(w):  # [G, K] -> [KC, 128, G] : wT[kc, p, g] = w[g, 128*kc+p]
        w = np.asarray(w, dtype=np.float32)
        return np.ascontiguousarray(w.T.reshape(KC, 128, G))

    def arrange_whh_coltile(w):
        # [G, K] -> [KC, 128, G] with gate columns permuted for the 4x
        # col-tiled scan matmul + 32x32 stream transpose:
        # arranged col (a*384 + 32*j + u) holds std col
        #   512*(j//4) + 128*(j%4) + 32*a + u
        w = np.asarray(w, dtype=np.float32)
        acol = np.arange(G)
        a, f = acol // 384, acol % 384
        j, u = f // 32, f % 32
        std = 512 * (j // 4) + 128 * (j % 4) + 32 * a + u
        return np.ascontiguousarray(w.T.reshape(KC, 128, G)[:, :, std])

    def arrange_gxbias(b_ih, b_hh):  # [128, MC]
        b = np.asarray(b_ih, dtype=np.float32).copy()
        b[:2 * H] += np.asarray(b_hh, dtype=np.float32)[:2 * H]
        return np.ascontiguousarray(b.reshape(MC, 128).T)

    def arrange_bhnpst(b_hh):
        # [128, 128] psum-layout n-gate bias: bhnpst[32a+b, 32jj+u] =
        # b_hh[2H + 128jj + 32a + u]  (independent of b; pad rows too)
        b = np.asarray(b_hh, dtype=np.float32)[2 * H:]
        out = np.empty((128, 128), np.float32)
        p = np.arange(128)
        a = p >> 5
        jj, u = np.meshgrid(np.arange(4), np.arange(32), indexing='ij')
        cols = (128 * jj + u).reshape(-1)  # [128] std col offset per (jj,u)
        for pi in range(128):
            out[pi, :] = b[cols + 32 * a[pi]]
        import ml_dtypes
        return np.ascontiguousarray(out.astype(ml_dtypes.bfloat16))

    import ml_dtypes
    per_dir = []
    for (w_ih, w_hh, b_ih, b_hh) in ((w_ih_lr, w_hh_lr, b_ih_lr, b_hh_lr),
                                     (w_ih_rl, w_hh_rl, b_ih_rl, b_hh_rl)):
        arr = arrange_whh_coltile(w_hh)
        per_dir.append({
            'whhT': arr.astype(ml_dtypes.bfloat16),
            'wihT': arrange_w(w_ih),
            'gxbias': arrange_gxbias(b_ih, b_hh),
            'bhnpst': arrange_bhnpst(b_hh),
        })
    in_maps = []
    for c in range(n_cores):
        bg, d = c >> 1, c & 1
        m = dict(per_dir[d])
        fs = feats[bg * B_SH:(bg + 1) * B_SH]
        if d == 1:
            fs = fs[:, ::-1, :]
        m['feats'] = np.ascontiguousarray(fs)
        in_maps.append(m)
    return in_maps


@with_exitstack
def gru_core_kernel(ctx: ExitStack, tc: tile.TileContext,
                    out_ap: bass.AP, feats: bass.AP, whhT: bass.AP,
                    wihT: bass.AP, gxbias: bass.AP,
                    bhnpst: bass.AP, L: int, T: int):
    nc = tc.nc
    NCH = L // T
    NTG = T // TG            # 8-token groups per chunk
    TOK = T * B_SH           # tokens per chunk (512)

    singles = ctx.enter_context(tc.tile_pool(name="singles", bufs=1))
    xpool = ctx.enter_context(tc.tile_pool(name="xpool", bufs=2))
    xtpool = ctx.enter_context(tc.tile_pool(name="xtpool", bufs=2))
    gxnpool = ctx.enter_context(tc.tile_pool(name="gxnpool", bufs=2))
    histpool = ctx.enter_context(tc.tile_pool(name="histpool", bufs=2))
    outpool = ctx.enter_context(tc.tile_pool(name="outpool", bufs=2))
    scratch = ctx.enter_context(tc.tile_pool(name="scratch", bufs=3))
    scan_ps = ctx.enter_context(tc.tile_pool(name="scan_ps", bufs=2, space="PSUM"))
    proj_ps = ctx.enter_context(tc.tile_pool(name="proj_ps", bufs=2, space="PSUM"))
    tr_ps = ctx.enter_context(tc.tile_pool(name="tr_ps", bufs=2, space="PSUM"))

    ident = singles.tile([128, 128], F32, tag="ident", name="ident")
    make_identity(nc, ident)
    # bf16 identity: stationary for the psum-init "copy" matmuls.
    # id32b stacks four 32x32 identities so each 32-partition block can be
    # a matmul stationary with lhsT/rhs sharing the same base partition.
    identb = singles.tile([128, 128], BF16, tag="identb", name="identb")
    nc.scalar.copy(out=identb[:], in_=ident[:])

    # --- persistent weights / biases in SBUF ---
    whh_sb = singles.tile([128, KC, G], BF16, tag="whh", name="whh")
    nc.sync.dma_start(whh_sb[:], whhT.rearrange("kc p g -> p kc g"))
    wih_sb = singles.tile([128, KC, G], F32R, tag="wih", name="wih")
    nc.sync.dma_start(wih_sb[:], wihT.rearrange("kc p g -> p kc g"))
    gxb_sb = singles.tile([128, MC], F32, tag="gxb", name="gxb")
    nc.sync.dma_start(gxb_sb[:], gxbias)
    bhnp_sb = singles.tile([128, 128], BF16, tag="bhnp", name="bhnp")
    nc.sync.dma_start(bhnp_sb[:], bhnpst)
    # stationary for col-tiled scan matmul: 32 cols (B_SH real + zero pad)
    hTb = singles.tile([128, KC, 32], BF16, tag="hTb", name="hTb")
    nc.vector.memset(hTb[:], 0.0)
    # rz-gate input projections in psum-prewrite layout: gx_rz[p, g, t, j, u]
    # = gx[b=u, token (g,t), gate col 128*j + p]; u lanes 16:32 zero pad
    # (memset once). Manual double buffer: proj of chunk c writes gx_rz[c % 2].
    gx_rz = []
    for i in range(2):
        gr = singles.tile([128, NTG, TG, 8, 32], BF16, tag=f"gxrz{i}",
                          name=f"gxrz{i}")
        nc.vector.memset(gr[:], 0.0)
        gx_rz.append(gr)
    # block-transposed gx staging (psum layout); filled by per-chunk bulk
    # transposes, consumed by the per-step psum-init identity matmuls
    gx_ps = []
    for i in range(2):
        gp = singles.tile([128, NTG, TG, 8, 32], BF16, tag=f"gxps{i}",
                          name=f"gxps{i}")
        nc.vector.memset(gp[:], 0.0)
        gx_ps.append(gp)

    def copy_on(i, out, in_):
        eng = (nc.vector, nc.scalar)[i % 2]
        if eng is nc.scalar:
            eng.copy(out=out, in_=in_)
        else:
            eng.tensor_copy(out=out, in_=in_)

    def biasadd_on(i, out, in0, scalar1):
        eng = (nc.vector, nc.scalar)[i % 2]
        if eng is nc.scalar:
            eng.add(out=out, in_=in0, add=scalar1)
        else:
            eng.tensor_scalar(out=out, in0=in0, scalar1=scalar1, scalar2=None,
                              op0=mybir.AluOpType.add)

    def make_proj(c):
        """Build (gx_rz buf, gx_n tile) + emission thunks (DMA, transpose,
        proj matmul). Thunks are interleaved with the previous chunk's scan
        steps so proj matmuls fill the PE's dependency-wait gaps."""
        w0 = c * T
        grz = gx_rz[c % 2]
        gps = gx_ps[c % 2]
        xs = xpool.tile([128, NTG, IN], F32, tag="xstage", name="xstage")
        xT = xtpool.tile([128, KC, TOK], F32R, tag="xT", name="xT")
        gxn = gxnpool.tile([128, 4, NTG, B_SH, TG], BF16, tag="gxn", name="gxn")
        thunks = []

        def dma_thunk(b):
            # one DMA per batch row, clean APs: SBUF partitions [8b, 8b+8),
            # DRAM token dim split (g t) then permuted to match [t, g, i]
            def f():
                nc.sync.dma_start(
                    out=xs[TG * b:TG * (b + 1), :, :],
                    in_=feats[b, w0:w0 + T, :].rearrange(
                        "(g t) i -> t g i", t=TG))
            return f

        def tr_thunk(g, fc):
            def f():
                tp = tr_ps.tile([128, 128], F32, tag="tp", name="tp")
                nc.tensor.transpose(tp[:], xs[:, g, 128 * fc:128 * (fc + 1)], ident[:])
                copy_on(g * KC + fc, xT[:, fc, 128 * g:128 * (g + 1)], tp[:])
            return f

        def mm_thunk(mc):
            def f():
                pj = proj_ps.tile([128, TOK], F32, tag="pj", name="pj")
                for kc in range(KC):
                    nc.tensor.matmul(
                        pj[:],
                        lhsT=wih_sb[:, kc, 128 * mc:128 * (mc + 1)],
                        rhs=xT[:, kc, :],
                        start=(kc == 0), stop=(kc == KC - 1))
                if mc < 8:
                    dst = grz[:, :, :, mc, 0:B_SH].rearrange("p g t b -> p g b t")
                else:
                    dst = gxn[:, mc - 8, :, :, :]
                biasadd_on(mc, dst,
                           pj[:].rearrange("p (g b t) -> p g b t", b=B_SH, t=TG),
                           gxb_sb[:, mc:mc + 1])
            return f

        def bulk_tr_thunk(g, half):
            # block-transpose four steps' worth of rz gx into psum layout
            t0, t1_ = 4 * half, 4 * (half + 1)
            def f():
                nc.vector.transpose(
                    out=gps[:, g, t0:t1_, :, :].rearrange("p t j u -> p (t j u)"),
                    in_=grz[:, g, t0:t1_, :, :].rearrange("p t j u -> p (t j u)"))
            return f

        for b in range(B_SH):
            thunks.append(dma_thunk(b))
        for g in range(NTG):
            for fc in range(KC):
                thunks.append(tr_thunk(g, fc))
        for mc in range(MC):
            thunks.append(mm_thunk(mc))
        for g in range(NTG):
            for half in range(2):
                thunks.append(bulk_tr_thunk(g, half))
        return (grz, gxn, gps), thunks

    def prewrite_rz(gxps_buf, s):
        """Initialize the step-s rz psum tile with gx via an identity
        matmul (start=True arms the psum accumulation group); the rz
        h-matmuls then accumulate on top (start=False). The block-transposed
        gx comes from the per-chunk bulk transposes in gx_ps."""
        g, t = s >> 3, s & 7
        # full-bank tile: psum accumulation zero-regions are per 2KB bank
        pst_rz = scan_ps.tile([128, 512], F32, tag="pstrz", name="pstrz")[:, 0:256]
        nc.tensor.matmul(pst_rz[:], lhsT=identb[:],
                         rhs=gxps_buf[:, g, t, :, :],
                         start=True, stop=False, skip_group_check=True)
        return pst_rz

    def prewrite_n(s):
        pst_n = scan_ps.tile([128, 512], F32, tag="pstn", name="pstn")[:, 0:128]
        nc.tensor.matmul(pst_n[:], lhsT=identb[:], rhs=bhnp_sb[:],
                         start=True, stop=False, skip_group_check=True)
        return pst_n

    def scan_step(gx_pair, nxt_gx_pair, histT, s, s_nxt, pst_pair, pst_box):
        # rz matmuls first (kc-outer), then n matmuls: the n block + the
        # gate chain overlap. All start=False (psum pre-written with gx/bhn).
        g, t = s >> 3, s & 7
        gxn = gx_pair[1]
        pst_rz, pst_n = pst_pair
        for kc in range(KC):
            for a in range(4):
                nc.tensor.matmul(
                    pst_rz[32 * a:32 * (a + 1), :],
                    lhsT=hTb[:, kc, :],
                    rhs=whh_sb[:, kc, 384 * a:384 * a + 256],
                    start=False, stop=(kc == KC - 1), skip_group_check=True,
                    tile_position=(0, 32 * a))
        for kc in range(KC):
            for a in range(4):
                nc.tensor.matmul(
                    pst_n[32 * a:32 * (a + 1), :],
                    lhsT=hTb[:, kc, :],
                    rhs=whh_sb[:, kc, 384 * a + 256:384 * (a + 1)],
                    start=False, stop=(kc == KC - 1), skip_group_check=True,
                    tile_position=(0, 32 * a))
        # srz = gh_rz + gx_rz directly from the pre-written psum
        srz = scratch.tile([128, 8, 32], F32, tag="srz", name="srz")
        nc.vector.transpose(out=srz[:].rearrange("p j u -> p (j u)"),
                            in_=pst_rz[:])
        r_t = scratch.tile([128, 4, B_SH], F32, tag="r_t", name="r_t")
        nc.scalar.activation(out=r_t[:], in_=srz[:, 0:4, 0:B_SH],
                             func=mybir.ActivationFunctionType.Sigmoid)
        omz = scratch.tile([128, 4, B_SH], F32, tag="omz", name="omz")
        nc.scalar.activation(out=omz[:], in_=srz[:, 4:8, 0:B_SH],
                             func=mybir.ActivationFunctionType.Sigmoid,
                             scale=-1.0)
        # w = ghn + bhn (psum pre-write included bhn)
        w_t = scratch.tile([128, 4, 32], F32, tag="w_t", name="w_t")
        nc.vector.transpose(out=w_t[:].rearrange("p j u -> p (j u)"),
                            in_=pst_n[:])
        # off-chain: hmid = h - omz*h  (= z*h), using the OLD h
        t2 = scratch.tile([128, 4, B_SH], F32, tag="t2", name="t2")
        nc.gpsimd.tensor_tensor(out=t2[:], in0=hTb[:, :, 0:B_SH], in1=omz[:],
                                op=mybir.AluOpType.mult)
        hmid = scratch.tile([128, 4, B_SH], F32, tag="hmid", name="hmid")
        nc.gpsimd.tensor_tensor(out=hmid[:], in0=hTb[:, :, 0:B_SH], in1=t2[:],
                                op=mybir.AluOpType.subtract)
        # chain: v = w*r + gxn ; n = tanh(v) ; h' = hmid + omz*n
        v = scratch.tile([128, 4, B_SH], F32, tag="v", name="v")
        nc.vector.tensor_tensor(out=v[:], in0=w_t[:, :, 0:B_SH], in1=r_t[:],
                                op=mybir.AluOpType.mult)
        nc.vector.tensor_tensor(out=v[:], in0=v[:], in1=gxn[:, :, g, :, t],
                                op=mybir.AluOpType.add)
        n_t = scratch.tile([128, 4, B_SH], F32, tag="n_t", name="n_t")
        nc.scalar.activation(out=n_t[:], in_=v[:],
                             func=mybir.ActivationFunctionType.Tanh)
        if s_nxt is not None:
            nxt_n = prewrite_n(s_nxt)
        # tail split by kc-half: the next step's kc0/kc1 matmuls only need
        # the first half of h, so they can start while the second half of
        # the update still runs.
        t1 = scratch.tile([128, 4, B_SH], F32, tag="t1", name="t1")
        nc.vector.tensor_tensor(out=t1[:, 0:2, :], in0=n_t[:, 0:2, :],
                                in1=omz[:, 0:2, :], op=mybir.AluOpType.mult)
        nc.vector.tensor_tensor(out=hTb[:, 0:2, 0:B_SH], in0=hmid[:, 0:2, :],
                                in1=t1[:, 0:2, :], op=mybir.AluOpType.add)
        nc.vector.tensor_tensor(out=t1[:, 2:4, :], in0=n_t[:, 2:4, :],
                                in1=omz[:, 2:4, :], op=mybir.AluOpType.mult)
        nc.vector.tensor_tensor(out=hTb[:, 2:4, 0:B_SH], in0=hmid[:, 2:4, :],
                                in1=t1[:, 2:4, :], op=mybir.AluOpType.add)
        nc.gpsimd.tensor_copy(out=histT[:, :, g, :, t], in_=hTb[:, :, 0:B_SH])
        # NEXT step's rz psum pre-write: emitted last so the wait-queue
        # admits it only during the tanh window (not ahead of w-tr).
        if s_nxt is not None:
            pst_box.append((prewrite_rz(nxt_gx_pair[2], s_nxt), nxt_n))

    def out_chunk(c, histT):
        w0 = c * T
        ost = outpool.tile([128, NTG, H], F32, tag="ost", name="ost")
        for g in range(NTG):
            for kc in range(KC):
                tp = tr_ps.tile([128, 128], F32, tag="tp", name="tp")
                nc.tensor.transpose(
                    tp[:],
                    histT[:, kc, g, :, :].rearrange("p b t -> p (b t)"),
                    ident[:])
                copy_on(g * KC + kc, ost[:, g, 128 * kc:128 * (kc + 1)], tp[:])
        def odma(b):
            def f():
                nc.sync.dma_start(
                    out=out_ap[b, w0:w0 + T, :]
                    .rearrange("(g t) h -> t g h", t=TG),
                    in_=ost[TG * b:TG * (b + 1), :, :])
            return f
        return [odma(b) for b in range(B_SH)]

    # --- software-pipelined chunk loop ---
    gx_cur, ths = make_proj(0)
    for th in ths:
        th()
    pst_cur = (prewrite_rz(gx_cur[2], 0), prewrite_n(0))
    carry = []
    for c in range(NCH):
        if c + 1 < NCH:
            gx_next, pending = make_proj(c + 1)
        else:
            pending, gx_next = [], None
        pending = carry + pending
        carry = []
        hist = histpool.tile([128, KC, NTG, B_SH, TG], F32, tag="hist", name="hist")
        # drain proj thunks a couple of steps before the chunk ends: the
        # last scan step emits the next chunk's psum prewrite, which must
        # sit AFTER its producing proj ops in each engine queue.
        per = max(1, -(-len(pending) // (T - 8))) if pending else 0
        for t in range(T):
            if t == T - 1:
                while pending:
                    pending.pop(0)()
            if t + 1 < T:
                s_nxt, nxt_gx = t + 1, gx_cur
            elif c + 1 < NCH:
                s_nxt, nxt_gx = 0, gx_next
            else:
                s_nxt, nxt_gx = None, None
            box = []
            scan_step(gx_cur, nxt_gx, hist, t, s_nxt, pst_cur, box)
            if box:
                pst_cur = box[0]
            for _ in range(per):
                if pending:
                    pending.pop(0)()
        if c + 1 < NCH:
            carry = out_chunk(c, hist)
        else:
            for th in out_chunk(c, hist):
                th()
        gx_cur = gx_next


def build_nc(L=1024, T=32, num_devices=8, debug=False):
    from concourse import bacc
    nc = bacc.Bacc("TRN2", target_bir_lowering=False, debug=debug,
                   enable_asserts=True, num_devices=num_devices)
    feats = nc.dram_tensor("feats", [B_SH, L, IN], F32, kind="ExternalInput").ap()
    whhT = nc.dram_tensor("whhT", [KC, 128, G], BF16, kind="ExternalInput").ap()
    wihT = nc.dram_tensor("wihT", [KC, 128, G], F32R, kind="ExternalInput").ap()
    gxbias = nc.dram_tensor("gxbias", [128, MC], F32, kind="ExternalInput").ap()
    bhnpst = nc.dram_tensor("bhnpst", [128, 128], BF16, kind="ExternalInput").ap()
    out = nc.dram_tensor("out", [B_SH, L, H], F32, kind="ExternalOutput").ap()
    with tile.TileContext(nc) as tc:
        gru_core_kernel(tc, out, feats, whhT, wihT, gxbias, bhnpst, L, T)
    nc.compile()
    return nc


# ---------------------------------------------------------------------------
# Self-contained harness entry point: kernel(**inputs) -> np.ndarray
# ---------------------------------------------------------------------------

N_CORES = 8
L_FULL = 1024
T_CHUNK = 32

_STATE = {}


def _get_exec():
    if 'fn' in _STATE:
        return _STATE
    import jax
    from jax.sharding import Mesh, PartitionSpec
    from jax.experimental.shard_map import shard_map
    from concourse.bass2jax import (_bass_exec_p, install_neuronx_cc_hook,
                                    partition_id_tensor)

    nc = build_nc(L=L_FULL, T=T_CHUNK, num_devices=N_CORES)
    install_neuronx_cc_hook()
    partition_name = nc.partition_id_tensor.name if nc.partition_id_tensor else None

    in_names, out_names, out_avals = [], [], []
    for alloc in nc.m.functions[0].allocations:
        if not isinstance(alloc, mybir.MemoryLocationSet):
            continue
        name = alloc.memorylocations[0].name
        if alloc.kind == "ExternalInput":
            if name != partition_name:
                in_names.append(name)
        elif alloc.kind == "ExternalOutput":
            out_names.append(name)
            out_avals.append(jax.core.ShapedArray(
                tuple(alloc.tensor_shape), mybir.dt.np(alloc.dtype)))
    all_in_names = list(in_names) + list(out_names)
    if partition_name is not None:
        all_in_names.append(partition_name)

    def _body(*args):
        operands = list(args)
        if partition_name is not None:
            operands.append(partition_id_tensor())
        return tuple(_bass_exec_p.bind(
            *operands, out_avals=tuple(out_avals), in_names=tuple(all_in_names),
            out_names=tuple(out_names), lowering_input_output_aliases=(),
            sim_require_finite=True, sim_require_nnan=True, nc=nc))

    devices = jax.devices()[:N_CORES]
    mesh = Mesh(np.asarray(devices), ("core",))
    spec = PartitionSpec("core")
    n_in = len(in_names) + len(out_avals)
    fn = jax.jit(shard_map(_body, mesh=mesh, in_specs=(spec,) * n_in,
                           out_specs=(spec,) * len(out_names), check_rep=False),
                 keep_unused=True)
    _STATE.update(fn=fn, in_names=in_names, out_names=out_names,
                  out_avals=out_avals, mesh=mesh, spec=spec)
    return _STATE


def _stage_inputs(in_maps):
    import jax
    from jax.sharding import NamedSharding
    st = _get_exec()
    sh = NamedSharding(st['mesh'], st['spec'])
    args = []
    for nm in st['in_names']:
        a = np.concatenate([np.asarray(in_maps[c][nm]) for c in range(N_CORES)],
                           axis=0)
        args.append(jax.device_put(a, sh))
    for av in st['out_avals']:
        z = np.zeros((N_CORES * av.shape[0], *av.shape[1:]), av.dtype)
        args.append(jax.device_put(z, sh))
    return args


def _run(args):
    import jax
    st = _get_exec()
    outs = st['fn'](*args)
    jax.block_until_ready(outs)
    return outs


def kernel(feats, feats_mask, w_ih_lr, w_hh_lr, b_ih_lr, b_hh_lr,
           w_ih_rl, w_hh_rl, b_ih_rl, b_hh_rl):
    """Full-input bidirectional GRU on 8 NeuronCores (4 batch-groups x 2 dirs).

    feats_mask is all-ones for this problem spec and is not used on device.
    """
    in_maps = prep_inputs(feats, w_ih_lr, w_hh_lr, b_ih_lr, b_hh_lr,
                          w_ih_rl, w_hh_rl, b_ih_rl, b_hh_rl, n_cores=N_CORES)
    args = _stage_inputs(in_maps)
    outs = _run(args)
    st = _STATE
    oi = st['out_names'].index('out')
    shards = np.asarray(outs[oi])  # [N_CORES*B_SH, L, H]
    B = N_CORES // 2 * B_SH
    full = np.empty((B, L_FULL, 2 * H), np.float32)
    for c in range(N_CORES):
        bg, d = c >> 1, c & 1
        piece = shards[c * B_SH:(c + 1) * B_SH]
        if d == 0:
            full[bg * B_SH:(bg + 1) * B_SH, :, 0:H] = piece
        else:
            full[bg * B_SH:(bg + 1) * B_SH, :, H:2 * H] = piece[:, ::-1, :]
    return full


# revision 3
# speedup vs baseline: 1.0129x; 1.0129x over previous
"""Bidirectional GRU Bass kernel for TRN2 — v2: direction-parallel sharding.

Problem: B=64, L=1024, IN=H=512, bidirectional GRU (torch GRUCell semantics),
mask = ones, output concat([lr, reversed(rl)], axis=2).

Sharding v2: 8 cores = 4 batch groups x 2 directions. Each core runs ONE
direction over B_SH=16 sequences. The rl cores receive time-reversed feats
(host-side flip) so the on-device program is identical SPMD; the host
re-reverses their output when assembling the full [64, L, 2H] tensor.

Per-core layout ("transposed domain"):
  - hidden state hTb[p, kc, b] = h[b, 128*kc + p]   (SBUF [128, 4, 32], bf16,
    cols 16:32 zero pad)
  - recurrent matmul: 4x col-tiled, stationary hTb, moving Whh (bf16),
    psum [128, 384]; 32x32 stream transpose into gate domain [128, 12, 32].
  - token partition order within an 8-token group g: p = 8*b + t.
"""

from contextlib import ExitStack

import numpy as np

import concourse.bass as bass
import concourse.mybir as mybir
import concourse.tile as tile
from concourse._compat import with_exitstack
from concourse.masks import make_identity

F32 = mybir.dt.float32
F32R = mybir.dt.float32r
BF16 = mybir.dt.bfloat16
F8E4 = mybir.dt.float8e4

IN = 512
H = 512
G = 3 * H  # 1536
KC = 4     # k chunks of 128 (contraction over H or IN)
MC = 12    # gate chunks of 128 (3H)
B_SH = 16  # batch per core (4 groups x 2 dirs over 8 cores)
TG = 8     # tokens per partition-group


def prep_inputs(feats, w_ih_lr, w_hh_lr, b_ih_lr, b_hh_lr,
                w_ih_rl, w_hh_rl, b_ih_rl, b_hh_rl, n_cores=8):
    """Host-side: shard feats over (batch-group, direction), arrange weights.

    Core c handles batch group c>>1 with direction c&1 (0=lr, 1=rl).
    rl cores get time-reversed feats so the device program is direction-free.
    """
    feats = np.asarray(feats, dtype=np.float32)

    def arrange_w(w):  # [G, K] -> [KC, 128, G] : wT[kc, p, g] = w[g, 128*kc+p]
        w = np.asarray(w, dtype=np.float32)
        return np.ascontiguousarray(w.T.reshape(KC, 128, G))

    def arrange_whh_coltile(w):
        # [G, K] -> [KC, 128, G] with gate columns permuted for the 4x
        # col-tiled scan matmul + 32x32 stream transpose:
        # arranged col (a*384 + 32*j + u) holds std col
        #   512*(j//4) + 128*(j%4) + 32*a + u
        w = np.asarray(w, dtype=np.float32)
        acol = np.arange(G)
        a, f = acol // 384, acol % 384
        j, u = f // 32, f % 32
        std = 512 * (j // 4) + 128 * (j % 4) + 32 * a + u
        return np.ascontiguousarray(w.T.reshape(KC, 128, G)[:, :, std])

    def arrange_gxbias(b_ih, b_hh):  # [128, MC]
        b = np.asarray(b_ih, dtype=np.float32).copy()
        b[:2 * H] += np.asarray(b_hh, dtype=np.float32)[:2 * H]
        return np.ascontiguousarray(b.reshape(MC, 128).T)

    def arrange_bhnpst(b_hh):
        # [128, 128] psum-layout n-gate bias: bhnpst[32a+b, 32jj+u] =
        # b_hh[2H + 128jj + 32a + u]  (independent of b; pad rows too)
        b = np.asarray(b_hh, dtype=np.float32)[2 * H:]
        out = np.empty((128, 128), np.float32)
        p = np.arange(128)
        a = p >> 5
        jj, u = np.meshgrid(np.arange(4), np.arange(32), indexing='ij')
        cols = (128 * jj + u).reshape(-1)  # [128] std col offset per (jj,u)
        for pi in range(128):
            out[pi, :] = b[cols + 32 * a[pi]]
        import ml_dtypes
        return np.ascontiguousarray(out.astype(ml_dtypes.bfloat16))

    import ml_dtypes
    per_dir = []
    for (w_ih, w_hh, b_ih, b_hh) in ((w_ih_lr, w_hh_lr, b_ih_lr, b_hh_lr),
                                     (w_ih_rl, w_hh_rl, b_ih_rl, b_hh_rl)):
        arr = arrange_whh_coltile(w_hh)
        per_dir.append({
            'whhT': arr.astype(ml_dtypes.bfloat16),
            'wihT': arrange_w(w_ih),
            'gxbias': arrange_gxbias(b_ih, b_hh),
            'bhnpst': arrange_bhnpst(b_hh),
        })
    in_maps = []
    for c in range(n_cores):
        bg, d = c >> 1, c & 1
        m = dict(per_dir[d])
        fs = feats[bg * B_SH:(bg + 1) * B_SH]
        if d == 1:
            fs = fs[:, ::-1, :]
        m['feats'] = np.ascontiguousarray(fs)
        in_maps.append(m)
    return in_maps


@with_exitstack
def gru_core_kernel(ctx: ExitStack, tc: tile.TileContext,
                    out_ap: bass.AP, feats: bass.AP, whhT: bass.AP,
                    wihT: bass.AP, gxbias: bass.AP,
                    bhnpst: bass.AP, L: int, T: int):
    nc = tc.nc
    NCH = L // T
    NTG = T // TG            # 8-token groups per chunk
    TOK = T * B_SH           # tokens per chunk (512)

    singles = ctx.enter_context(tc.tile_pool(name="singles", bufs=1))
    xpool = ctx.enter_context(tc.tile_pool(name="xpool", bufs=2))
    xtpool = ctx.enter_context(tc.tile_pool(name="xtpool", bufs=2))
    gxnpool = ctx.enter_context(tc.tile_pool(name="gxnpool", bufs=2))
    histpool = ctx.enter_context(tc.tile_pool(name="histpool", bufs=2))
    outpool = ctx.enter_context(tc.tile_pool(name="outpool", bufs=2))
    scratch = ctx.enter_context(tc.tile_pool(name="scratch", bufs=3))
    scan_ps = ctx.enter_context(tc.tile_pool(name="scan_ps", bufs=2, space="PSUM"))
    proj_ps = ctx.enter_context(tc.tile_pool(name="proj_ps", bufs=2, space="PSUM"))
    tr_ps = ctx.enter_context(tc.tile_pool(name="tr_ps", bufs=2, space="PSUM"))

    ident = singles.tile([128, 128], F32, tag="ident", name="ident")
    make_identity(nc, ident)
    # bf16 identity: stationary for the psum-init "copy" matmuls.
    # id32b stacks four 32x32 identities so each 32-partition block can be
    # a matmul stationary with lhsT/rhs sharing the same base partition.
    identb = singles.tile([128, 128], BF16, tag="identb", name="identb")
    nc.scalar.copy(out=identb[:], in_=ident[:])

    # --- persistent weights / biases in SBUF ---
    whh_sb = singles.tile([128, KC, G], BF16, tag="whh", name="whh")
    nc.sync.dma_start(whh_sb[:], whhT.rearrange("kc p g -> p kc g"))
    wih_sb = singles.tile([128, KC, G], F32R, tag="wih", name="wih")
    nc.sync.dma_start(wih_sb[:], wihT.rearrange("kc p g -> p kc g"))
    gxb_sb = singles.tile([128, MC], F32, tag="gxb", name="gxb")
    nc.sync.dma_start(gxb_sb[:], gxbias)
    bhnp_sb = singles.tile([128, 128], BF16, tag="bhnp", name="bhnp")
    nc.sync.dma_start(bhnp_sb[:], bhnpst)
    # stationary for col-tiled scan matmul: 32 cols (B_SH real + zero pad)
    hTb = singles.tile([128, KC, 32], BF16, tag="hTb", name="hTb")
    nc.vector.memset(hTb[:], 0.0)
    # rz-gate input projections in psum-prewrite layout: gx_rz[p, g, t, j, u]
    # = gx[b=u, token (g,t), gate col 128*j + p]; u lanes 16:32 zero pad
    # (memset once). Manual double buffer: proj of chunk c writes gx_rz[c % 2].
    gx_rz = []
    for i in range(2):
        gr = singles.tile([128, NTG, TG, 8, 32], BF16, tag=f"gxrz{i}",
                          name=f"gxrz{i}")
        nc.vector.memset(gr[:], 0.0)
        gx_rz.append(gr)
    # block-transposed gx staging (psum layout); filled by per-chunk bulk
    # transposes, consumed by the per-step psum-init identity matmuls
    gx_ps = []
    for i in range(2):
        gp = singles.tile([128, NTG, TG, 8, 32], BF16, tag=f"gxps{i}",
                          name=f"gxps{i}")
        nc.vector.memset(gp[:], 0.0)
        gx_ps.append(gp)

    def copy_on(i, out, in_):
        eng = (nc.vector, nc.scalar)[i % 2]
        if eng is nc.scalar:
            eng.copy(out=out, in_=in_)
        else:
            eng.tensor_copy(out=out, in_=in_)

    def biasadd_on(i, out, in0, scalar1):
        eng = (nc.vector, nc.scalar)[i % 2]
        if eng is nc.scalar:
            eng.add(out=out, in_=in0, add=scalar1)
        else:
            eng.tensor_scalar(out=out, in0=in0, scalar1=scalar1, scalar2=None,
                              op0=mybir.AluOpType.add)

    def make_proj(c):
        """Build (gx_rz buf, gx_n tile) + emission thunks (DMA, transpose,
        proj matmul). Thunks are interleaved with the previous chunk's scan
        steps so proj matmuls fill the PE's dependency-wait gaps."""
        w0 = c * T
        grz = gx_rz[c % 2]
        gps = gx_ps[c % 2]
        xs = xpool.tile([128, NTG, IN], F32, tag="xstage", name="xstage")
        xT = xtpool.tile([128, KC, TOK], F32R, tag="xT", name="xT")
        gxn = gxnpool.tile([128, 4, NTG, B_SH, TG], BF16, tag="gxn", name="gxn")
        thunks = []

        def dma_thunk(b):
            # one DMA per batch row, clean APs: SBUF partitions [8b, 8b+8),
            # DRAM token dim split (g t) then permuted to match [t, g, i]
            def f():
                nc.sync.dma_start(
                    out=xs[TG * b:TG * (b + 1), :, :],
                    in_=feats[b, w0:w0 + T, :].rearrange(
                        "(g t) i -> t g i", t=TG))
            return f

        def tr_thunk(g, fc):
            def f():
                tp = tr_ps.tile([128, 128], F32, tag="tp", name="tp")
                nc.tensor.transpose(tp[:], xs[:, g, 128 * fc:128 * (fc + 1)], ident[:])
                copy_on(g * KC + fc, xT[:, fc, 128 * g:128 * (g + 1)], tp[:])
            return f

        def mm_thunk(mc):
            def f():
                pj = proj_ps.tile([128, TOK], F32, tag="pj", name="pj")
                for kc in range(KC):
                    nc.tensor.matmul(
                        pj[:],
                        lhsT=wih_sb[:, kc, 128 * mc:128 * (mc + 1)],
                        rhs=xT[:, kc, :],
                        start=(kc == 0), stop=(kc == KC - 1))
                if mc < 8:
                    dst = grz[:, :, :, mc, 0:B_SH].rearrange("p g t b -> p g b t")
                else:
                    dst = gxn[:, mc - 8, :, :, :]
                biasadd_on(mc, dst,
                           pj[:].rearrange("p (g b t) -> p g b t", b=B_SH, t=TG),
                           gxb_sb[:, mc:mc + 1])
            return f

        def bulk_tr_thunk(g, half):
            # block-transpose four steps' worth of rz gx into psum layout
            t0, t1_ = 4 * half, 4 * (half + 1)
            def f():
                nc.vector.transpose(
                    out=gps[:, g, t0:t1_, :, :].rearrange("p t j u -> p (t j u)"),
                    in_=grz[:, g, t0:t1_, :, :].rearrange("p t j u -> p (t j u)"))
            return f

        for b in range(B_SH):
            thunks.append(dma_thunk(b))
        for g in range(NTG):
            for fc in range(KC):
                thunks.append(tr_thunk(g, fc))
        for mc in range(MC):
            thunks.append(mm_thunk(mc))
        for g in range(NTG):
            for half in range(2):
                thunks.append(bulk_tr_thunk(g, half))
        return (grz, gxn, gps), thunks

    def prewrite_rz(gxps_buf, s):
        """Initialize the step-s rz psum tile with gx via an identity
        matmul (start=True arms the psum accumulation group); the rz
        h-matmuls then accumulate on top (start=False). The block-transposed
        gx comes from the per-chunk bulk transposes in gx_ps."""
        g, t = s >> 3, s & 7
        # full-bank tile: psum accumulation zero-regions are per 2KB bank
        pst_rz = scan_ps.tile([128, 512], F32, tag="pstrz", name="pstrz")[:, 0:256]
        nc.tensor.matmul(pst_rz[:], lhsT=identb[:],
                         rhs=gxps_buf[:, g, t, :, :],
                         start=True, stop=False, skip_group_check=True)
        return pst_rz

    def prewrite_n(s):
        pst_n = scan_ps.tile([128, 512], F32, tag="pstn", name="pstn")[:, 0:128]
        nc.tensor.matmul(pst_n[:], lhsT=identb[:], rhs=bhnp_sb[:],
                         start=True, stop=False, skip_group_check=True)
        return pst_n

    def scan_step(gx_pair, nxt_gx_pair, histT, s, s_nxt, pst_pair, pst_box):
        # rz matmuls first (kc-outer), then n matmuls: the n block + the
        # gate chain overlap. All start=False (psum pre-written with gx/bhn).
        g, t = s >> 3, s & 7
        gxn = gx_pair[1]
        pst_rz, pst_n = pst_pair
        for kc in range(KC):
            for a in range(4):
                nc.tensor.matmul(
                    pst_rz[32 * a:32 * (a + 1), :],
                    lhsT=hTb[:, kc, :],
                    rhs=whh_sb[:, kc, 384 * a:384 * a + 256],
                    start=False, stop=(kc == KC - 1), skip_group_check=True,
                    tile_position=(0, 32 * a))
        for kc in range(KC):
            for a in range(4):
                nc.tensor.matmul(
                    pst_n[32 * a:32 * (a + 1), :],
                    lhsT=hTb[:, kc, :],
                    rhs=whh_sb[:, kc, 384 * a + 256:384 * (a + 1)],
                    start=False, stop=(kc == KC - 1), skip_group_check=True,
                    tile_position=(0, 32 * a))
        # srz = gh_rz + gx_rz directly from the pre-written psum
        srz = scratch.tile([128, 8, 32], F32, tag="srz", name="srz")
        nc.vector.transpose(out=srz[:].rearrange("p j u -> p (j u)"),
                            in_=pst_rz[:])
        r_t = scratch.tile([128, 4, B_SH], F32, tag="r_t", name="r_t")
        nc.scalar.activation(out=r_t[:], in_=srz[:, 0:4, 0:B_SH],
                             func=mybir.ActivationFunctionType.Sigmoid)
        omz = scratch.tile([128, 4, B_SH], F32, tag="omz", name="omz")
        nc.scalar.activation(out=omz[:], in_=srz[:, 4:8, 0:B_SH],
                             func=mybir.ActivationFunctionType.Sigmoid,
                             scale=-1.0)
        # w = ghn + bhn (psum pre-write included bhn)
        w_t = scratch.tile([128, 4, 32], F32, tag="w_t", name="w_t")
        nc.vector.transpose(out=w_t[:].rearrange("p j u -> p (j u)"),
                            in_=pst_n[:])
        # off-chain: hmid = h - omz*h  (= z*h), using the OLD h
        t2 = scratch.tile([128, 4, B_SH], F32, tag="t2", name="t2")
        nc.gpsimd.tensor_tensor(out=t2[:], in0=hTb[:, :, 0:B_SH], in1=omz[:],
                                op=mybir.AluOpType.mult)
        hmid = scratch.tile([128, 4, B_SH], F32, tag="hmid", name="hmid")
        nc.gpsimd.tensor_tensor(out=hmid[:], in0=hTb[:, :, 0:B_SH], in1=t2[:],
                                op=mybir.AluOpType.subtract)
        # chain: v = w*r + gxn ; n = tanh(v) ; h' = hmid + omz*n
        v = scratch.tile([128, 4, B_SH], F32, tag="v", name="v")
        nc.vector.tensor_tensor(out=v[:], in0=w_t[:, :, 0:B_SH], in1=r_t[:],
                                op=mybir.AluOpType.mult)
        nc.vector.tensor_tensor(out=v[:], in0=v[:], in1=gxn[:, :, g, :, t],
                                op=mybir.AluOpType.add)
        n_t = scratch.tile([128, 4, B_SH], F32, tag="n_t", name="n_t")
        nc.scalar.activation(out=n_t[:], in_=v[:],
                             func=mybir.ActivationFunctionType.Tanh)
        if s_nxt is not None:
            nxt_n = prewrite_n(s_nxt)
        # tail split by kc-half: the next step's kc0/kc1 matmuls only need
        # the first half of h, so they can start while the second half of
        # the update still runs.
        t1 = scratch.tile([128, 4, B_SH], F32, tag="t1", name="t1")
        nc.vector.tensor_tensor(out=t1[:, 0:2, :], in0=n_t[:, 0:2, :],
                                in1=omz[:, 0:2, :], op=mybir.AluOpType.mult)
        nc.vector.tensor_tensor(out=hTb[:, 0:2, 0:B_SH], in0=hmid[:, 0:2, :],
                                in1=t1[:, 0:2, :], op=mybir.AluOpType.add)
        nc.vector.tensor_tensor(out=t1[:, 2:4, :], in0=n_t[:, 2:4, :],
                                in1=omz[:, 2:4, :], op=mybir.AluOpType.mult)
        nc.vector.tensor_tensor(out=hTb[:, 2:4, 0:B_SH], in0=hmid[:, 2:4, :],
                                in1=t1[:, 2:4, :], op=mybir.AluOpType.add)
        nc.gpsimd.tensor_copy(out=histT[:, :, g, :, t], in_=hTb[:, :, 0:B_SH])
        # NEXT step's rz psum pre-write: emitted last so the wait-queue
        # admits it only during the tanh window (not ahead of w-tr).
        if s_nxt is not None:
            pst_box.append((prewrite_rz(nxt_gx_pair[2], s_nxt), nxt_n))

    def out_chunk(c, histT):
        # everything returned as thunks: the transposes, copies and DMAs
        # interleave into the NEXT chunk's scan steps (hist/ost double
        # buffers keep the data alive), avoiding a boundary pileup that
        # stalls the PE long enough to drop its p-state.
        w0 = c * T
        ost = outpool.tile([128, NTG, H], F32, tag="ost", name="ost")
        def otr(g, kc):
            def f():
                tp = tr_ps.tile([128, 128], F32, tag="tp", name="tp")
                nc.tensor.transpose(
                    tp[:],
                    histT[:, kc, g, :, :].rearrange("p b t -> p (b t)"),
                    ident[:])
                copy_on(g * KC + kc, ost[:, g, 128 * kc:128 * (kc + 1)], tp[:])
            return f
        def odma(b):
            def f():
                nc.sync.dma_start(
                    out=out_ap[b, w0:w0 + T, :]
                    .rearrange("(g t) h -> t g h", t=TG),
                    in_=ost[TG * b:TG * (b + 1), :, :])
            return f
        return ([otr(g, kc) for g in range(NTG) for kc in range(KC)]
                + [odma(b) for b in range(B_SH)])

    # --- software-pipelined chunk loop ---
    gx_cur, ths = make_proj(0)
    for th in ths:
        th()
    pst_cur = (prewrite_rz(gx_cur[2], 0), prewrite_n(0))
    carry = []
    for c in range(NCH):
        if c + 1 < NCH:
            gx_next, pending = make_proj(c + 1)
        else:
            pending, gx_next = [], None
        pending = carry + pending
        carry = []
        hist = histpool.tile([128, KC, NTG, B_SH, TG], F32, tag="hist", name="hist")
        # drain proj thunks a couple of steps before the chunk ends: the
        # last scan step emits the next chunk's psum prewrite, which must
        # sit AFTER its producing proj ops in each engine queue.
        per = max(1, -(-len(pending) // (T - 8))) if pending else 0
        for t in range(T):
            if t == T - 1:
                while pending:
                    pending.pop(0)()
            if t + 1 < T:
                s_nxt, nxt_gx = t + 1, gx_cur
            elif c + 1 < NCH:
                s_nxt, nxt_gx = 0, gx_next
            else:
                s_nxt, nxt_gx = None, None
            box = []
            scan_step(gx_cur, nxt_gx, hist, t, s_nxt, pst_cur, box)
            if box:
                pst_cur = box[0]
            for _ in range(per):
                if pending:
                    pending.pop(0)()
        if c + 1 < NCH:
            carry = out_chunk(c, hist)
        else:
            for th in out_chunk(c, hist):
                th()
        gx_cur = gx_next


def build_nc(L=1024, T=32, num_devices=8, debug=False):
    from concourse import bacc
    nc = bacc.Bacc("TRN2", target_bir_lowering=False, debug=debug,
                   enable_asserts=True, num_devices=num_devices)
    feats = nc.dram_tensor("feats", [B_SH, L, IN], F32, kind="ExternalInput").ap()
    whhT = nc.dram_tensor("whhT", [KC, 128, G], BF16, kind="ExternalInput").ap()
    wihT = nc.dram_tensor("wihT", [KC, 128, G], F32R, kind="ExternalInput").ap()
    gxbias = nc.dram_tensor("gxbias", [128, MC], F32, kind="ExternalInput").ap()
    bhnpst = nc.dram_tensor("bhnpst", [128, 128], BF16, kind="ExternalInput").ap()
    out = nc.dram_tensor("out", [B_SH, L, H], F32, kind="ExternalOutput").ap()
    with tile.TileContext(nc) as tc:
        gru_core_kernel(tc, out, feats, whhT, wihT, gxbias, bhnpst, L, T)
    nc.compile()
    return nc


# ---------------------------------------------------------------------------
# Self-contained harness entry point: kernel(**inputs) -> np.ndarray
# ---------------------------------------------------------------------------

N_CORES = 8
L_FULL = 1024
T_CHUNK = 32

_STATE = {}


def _get_exec():
    if 'fn' in _STATE:
        return _STATE
    import jax
    from jax.sharding import Mesh, PartitionSpec
    from jax.experimental.shard_map import shard_map
    from concourse.bass2jax import (_bass_exec_p, install_neuronx_cc_hook,
                                    partition_id_tensor)

    nc = build_nc(L=L_FULL, T=T_CHUNK, num_devices=N_CORES)
    install_neuronx_cc_hook()
    partition_name = nc.partition_id_tensor.name if nc.partition_id_tensor else None

    in_names, out_names, out_avals = [], [], []
    for alloc in nc.m.functions[0].allocations:
        if not isinstance(alloc, mybir.MemoryLocationSet):
            continue
        name = alloc.memorylocations[0].name
        if alloc.kind == "ExternalInput":
            if name != partition_name:
                in_names.append(name)
        elif alloc.kind == "ExternalOutput":
            out_names.append(name)
            out_avals.append(jax.core.ShapedArray(
                tuple(alloc.tensor_shape), mybir.dt.np(alloc.dtype)))
    all_in_names = list(in_names) + list(out_names)
    if partition_name is not None:
        all_in_names.append(partition_name)

    def _body(*args):
        operands = list(args)
        if partition_name is not None:
            operands.append(partition_id_tensor())
        return tuple(_bass_exec_p.bind(
            *operands, out_avals=tuple(out_avals), in_names=tuple(all_in_names),
            out_names=tuple(out_names), lowering_input_output_aliases=(),
            sim_require_finite=True, sim_require_nnan=True, nc=nc))

    devices = jax.devices()[:N_CORES]
    mesh = Mesh(np.asarray(devices), ("core",))
    spec = PartitionSpec("core")
    n_in = len(in_names) + len(out_avals)
    fn = jax.jit(shard_map(_body, mesh=mesh, in_specs=(spec,) * n_in,
                           out_specs=(spec,) * len(out_names), check_rep=False),
                 keep_unused=True)
    _STATE.update(fn=fn, in_names=in_names, out_names=out_names,
                  out_avals=out_avals, mesh=mesh, spec=spec)
    return _STATE


def _stage_inputs(in_maps):
    import jax
    from jax.sharding import NamedSharding
    st = _get_exec()
    sh = NamedSharding(st['mesh'], st['spec'])
    args = []
    for nm in st['in_names']:
        a = np.concatenate([np.asarray(in_maps[c][nm]) for c in range(N_CORES)],
                           axis=0)
        args.append(jax.device_put(a, sh))
    for av in st['out_avals']:
        z = np.zeros((N_CORES * av.shape[0], *av.shape[1:]), av.dtype)
        args.append(jax.device_put(z, sh))
    return args


def _run(args):
    import jax
    st = _get_exec()
    outs = st['fn'](*args)
    jax.block_until_ready(outs)
    return outs


def kernel(feats, feats_mask, w_ih_lr, w_hh_lr, b_ih_lr, b_hh_lr,
           w_ih_rl, w_hh_rl, b_ih_rl, b_hh_rl):
    """Full-input bidirectional GRU on 8 NeuronCores (4 batch-groups x 2 dirs).

    feats_mask is all-ones for this problem spec and is not used on device.
    """
    in_maps = prep_inputs(feats, w_ih_lr, w_hh_lr, b_ih_lr, b_hh_lr,
                          w_ih_rl, w_hh_rl, b_ih_rl, b_hh_rl, n_cores=N_CORES)
    args = _stage_inputs(in_maps)
    outs = _run(args)
    st = _STATE
    oi = st['out_names'].index('out')
    shards = np.asarray(outs[oi])  # [N_CORES*B_SH, L, H]
    B = N_CORES // 2 * B_SH
    full = np.empty((B, L_FULL, 2 * H), np.float32)
    for c in range(N_CORES):
        bg, d = c >> 1, c & 1
        piece = shards[c * B_SH:(c + 1) * B_SH]
        if d == 0:
            full[bg * B_SH:(bg + 1) * B_SH, :, 0:H] = piece
        else:
            full[bg * B_SH:(bg + 1) * B_SH, :, H:2 * H] = piece[:, ::-1, :]
    return full
